# revision 1
# baseline (speedup 1.0000x reference)
"""AQT int8-symmetric quantized dot_general on 8 Trainium2 NeuronCores. v5

Computes the equivalent of (AQT default int8 config):
    q_lhs, ls = quantize(lhs, axis=K)   # per-row abs-max/127 scales
    q_rhs, rs = quantize(rhs, axis=K)   # per-col abs-max/127 scales
    out = (q_lhs @ q_rhs) * ls * rs     # int32 accumulate, f32 dequant

Sharding: data-parallel over the flattened batch*seq rows of lhs (4096 rows
per core); rhs replicated. No collectives.

v5: one flat software-pipelined stream across all repeat bodies.
  - scales for group g+1 are computed a full group ahead; quant runs two
    tiles ahead of the matmuls; lhs loads three groups ahead. This removes
    the group-boundary stall (reduce->recip->quant->x-bar serial chain).
  - rhs prep (PE transposes + per-column quant) for body r+1 is injected
    into body r's tail so the PE instruction stream never idles between
    bodies.
  - PSUM drains lag 4 matmul units and alternate ACT/DVE; both dequant
    scales are folded into the bf16 operands so drains are plain copies.
  - output is fp16 (halves out-traffic); host upcasts to f32.
"""

import sys
from contextlib import ExitStack

import numpy as np

for _p in ("/root/.axon_site/_ro/trn_rl_repo", "/opt/trn_rl_repo"):
    if _p not in sys.path:
        sys.path.append(_p)

import concourse.bass as bass
import concourse.tile as tile
from concourse import bacc, bass_isa, mybir
from concourse.bass_utils import run_bass_kernel_spmd
from concourse.masks import make_identity

N_CORES = 8
K = 1024
N = 1024
M_FULL = 4 * 8192
M_SHARD = M_FULL // N_CORES  # 4096

P = 128                      # partitions
KT = K // P                  # 8 k-chunks
NT = N // P                  # 8 n-chunks (for rhs transpose)
NF = 512                     # moving free dim / PSUM bank
NCH = N // NF                # 2 n-chunks for the main matmul

C_RNE = 12582912.0           # 1.5 * 2**23: (x + C) - C == round-half-even(x)
INV_QB = 1.0 / 127.0
FP32 = mybir.dt.float32
FP16 = mybir.dt.float16
BF16 = mybir.dt.bfloat16
FX = mybir.AxisListType.X

import os
GRP = 4        # m-tiles per load group
QLOOK = int(os.environ.get("V_QLOOK", "3"))
GLOOK = 2      # load lookahead in groups
LAG = int(os.environ.get("V_LAG", "4"))
DRAIN = "alt"  # PSUM drain engine: alternate ACT/DVE by tile parity
RQ = "sync"    # rhs load queue
STQ = "scalar" # out store queue
CCOPY = "alt"  # chain PSUM-copy engines (chain mode only)
LBQ = "sync"   # lhs load queue
CXQ = "sync"   # chain x-bar queue (chain mode only)
CLEAD = int(os.environ.get("V_CLEAD", "10"))
LSPLIT = 0     # lhs loads as one DMA per group
RHS = "ar"     # rhs prep via gpsimd partition_all_reduce (no PE transposes)
SSPLIT = 0
XSPLIT = 0
LATELOAD = 0
SCAT = 0
QDR = "alt"
ARSPREAD = int(os.environ.get("V_ARSPREAD", "2"))  # ar pieces per tile


def _stream(tc: tile.TileContext, pl: dict, ident, out: bass.AP,
            lhs: bass.AP, rhs: bass.AP, m_shard: int, repeats: int):
    nc = tc.nc
    mt = m_shard // P
    grp = GRP
    assert mt % grp == 0 and mt >= 2 * NT
    ng = mt // grp
    tpern = NF // P

    Rs, QRSs, lbs, scs, obs, qtss = {}, {}, {}, {}, {}, {}
    pending = []

    def load_R(rep):
        R = pl["rhsq"].tile([P, KT, N], FP32, tag="R", name="R")
        rview = rhs.rearrange("(kt p) n -> p kt n", p=P)
        for rq in range(KT):
            (nc.sync if RQ == "sync" else nc.gpsimd).dma_start(
                R[:, rq:rq + 1, :], rview[:, rq:rq + 1, :])
        Rs[rep] = R
        QRSs[rep] = [
            pl["qrs"].tile([P, KT, NF], BF16, tag=f"qrs{nj}", name=f"qrs{nj}")
            for nj in range(NCH)]

    def chain(rep, t):
        # PE-transpose n-tile t: rt[p, k] = rhs[k, t*P + p]; quantize its
        # rows (original rhs columns, scale folded); x-bar back to [k, n].
        R = Rs[rep]
        rt = pl["rtrow"].tile([P, K], FP32, tag="rt", name="rt")
        for h in range(KT // 4):
            tps = pl["rpsum"].tile([P, 4 * P], FP32, tag="rtp", name="rtp")
            for q in range(4):
                j = 4 * h + q
                nc.tensor.transpose(tps[:, q * P:(q + 1) * P],
                                    R[:, j, t * P:(t + 1) * P], ident[:])
            if CCOPY == "pool":
                nc.gpsimd.tensor_copy(rt[:, h * 4 * P:(h + 1) * 4 * P],
                                      tps[:])
            elif CCOPY == "dve" or (CCOPY == "alt" and h % 2 == 0):
                nc.vector.tensor_copy(rt[:, h * 4 * P:(h + 1) * 4 * P], tps[:])
            else:
                nc.scalar.copy(rt[:, h * 4 * P:(h + 1) * 4 * P], tps[:])
        am_r = pl["scales"].tile([P, 1], FP32, tag="am_r", name="am_r")
        nc.vector.tensor_reduce(am_r[:], rt[:], FX, mybir.AluOpType.max,
                                apply_absolute_value=True)
        s_r = pl["scales"].tile([P, 1], FP32, tag="s_r", name="s_r")
        nc.vector.tensor_scalar(s_r[:], am_r[:], 1e-30, INV_QB,
                                op0=mybir.AluOpType.max,
                                op1=mybir.AluOpType.mult)
        inv_r = pl["scales"].tile([P, 1], FP32, tag="inv_r", name="inv_r")
        nc.vector.reciprocal(inv_r[:], s_r[:])
        pr = pl["rquant"].tile([P, K], FP32, tag="pr", name="pr")
        nc.scalar.activation(pr[:], rt[:], mybir.ActivationFunctionType.Copy,
                             bias=C_RNE, scale=inv_r[:])
        qrs_t = pl["rquant"].tile([P, K], BF16, tag="qrs_t", name="qrs_t")
        nc.vector.tensor_scalar(qrs_t[:], pr[:], -C_RNE, s_r[:],
                                op0=mybir.AluOpType.add,
                                op1=mybir.AluOpType.mult)
        (nc.sync if CXQ == "sync" else nc.scalar).dma_start_transpose(
            QRSs[rep][t // tpern][:, :, (t % tpern) * P:(t % tpern + 1) * P],
            qrs_t[:])

    ar_state = {}

    def rhs_ar(rep, step):
        # steps 0..7: per-chunk partition absmax (Pool) + running max fold;
        # step 8: scale + reciprocal; steps 9..16: per-chunk quantize.
        R = Rs[rep]
        st = ar_state.setdefault(rep, {})
        if step < KT:
            c = step
            ar = pl["arp"].tile([P, N], FP32, tag="ar", name="ar")
            nc.gpsimd.partition_all_reduce(
                ar[:], R[:, c, :], channels=P,
                reduce_op=bass_isa.ReduceOp.absmax)
            if c == 0:
                st["M"] = ar
            else:
                m2 = pl["arm"].tile([P, N], FP32, tag="arm", name="arm")
                nc.vector.tensor_tensor(m2[:], st["M"][:], ar[:],
                                        mybir.AluOpType.max)
                st["M"] = m2
        elif step == KT:
            sc = pl["arsc"].tile([P, N], FP32, tag="arsc", name="arsc")
            nc.vector.tensor_scalar(sc[:], st["M"][:], 1e-30, INV_QB,
                                    op0=mybir.AluOpType.max,
                                    op1=mybir.AluOpType.mult)
            inv = pl["arin"].tile([P, N], FP32, tag="arin", name="arin")
            nc.vector.reciprocal(inv[:], sc[:])
            st["sc"], st["inv"] = sc, inv
        else:
            c = step - KT - 1
            t1 = pl["art1"].tile([P, N], FP32, tag="art1", name="art1")
            nc.vector.tensor_tensor(t1[:], R[:, c, :], st["inv"][:],
                                    mybir.AluOpType.mult)
            t2 = pl["art2"].tile([P, N], FP32, tag="art2", name="art2")
            nc.scalar.activation(t2[:], t1[:],
                                 mybir.ActivationFunctionType.Copy,
                                 bias=C_RNE, scale=1.0)
            t3 = pl["art3"].tile([P, N], FP32, tag="art3", name="art3")
            nc.scalar.activation(t3[:], t2[:],
                                 mybir.ActivationFunctionType.Copy,
                                 bias=-C_RNE, scale=1.0)
            for nj in range(NCH):
                sl = slice(nj * NF, (nj + 1) * NF)
                nc.vector.tensor_tensor(
                    QRSs[rep][nj][:, c, :], t3[:, sl], st["sc"][:, sl],
                    mybir.AluOpType.mult)

    NPREP = 2 * KT + 1  # rhs-prep pieces per body

    def load_group(rep, g):
        lb = pl["lload"].tile([P, grp, K], FP32, tag="lb", name="lb")
        _lbq = {"gpsimd": nc.gpsimd, "scalar": nc.scalar,
                "sync": nc.sync}[LBQ]
        view = lhs[g * grp * P:(g + 1) * grp * P, :].rearrange(
            "(t p) k -> p t k", p=P)
        if LSPLIT > 1:
            step = grp // LSPLIT
            for t in range(0, grp, step):
                _lbq.dma_start(lb[:, t:t + step, :], view[:, t:t + step, :])
        else:
            _lbq.dma_start(lb[:], view)
        lbs[(rep, g)] = lb

    def scales(rep, g):
        lb = lbs[(rep, g)]
        am = pl["scales"].tile([P, grp], FP32, tag="am", name="am")
        nc.vector.tensor_reduce(am[:], lb[:], FX, mybir.AluOpType.max,
                                apply_absolute_value=True)
        s = pl["scales"].tile([P, grp], FP32, tag="s", name="s")
        nc.vector.tensor_scalar(s[:], am[:], 1e-30, INV_QB,
                                op0=mybir.AluOpType.max,
                                op1=mybir.AluOpType.mult)
        inv = pl["scales"].tile([P, grp], FP32, tag="inv", name="inv")
        nc.vector.reciprocal(inv[:], s[:])
        scs[(rep, g)] = (s, inv)

    def quant(rep, u):
        g, ti = divmod(u, grp)
        lb = lbs[(rep, g)]
        s, inv = scs[(rep, g)]
        pi = pl["lpass"].tile([P, K], FP32, tag="pi", name="pi")
        nc.scalar.activation(pi[:], lb[:, ti, :],
                             mybir.ActivationFunctionType.Copy,
                             bias=C_RNE, scale=inv[:, ti:ti + 1])
        # qi = (pi - C) * s[ti]: integer-valued quant * folded dequant scale
        qi = pl["lq"].tile([P, K], BF16, tag="qi", name="qi")
        nc.vector.tensor_scalar(qi[:], pi[:], -C_RNE, s[:, ti:ti + 1],
                                op0=mybir.AluOpType.add,
                                op1=mybir.AluOpType.mult)
        qt = pl["lqt"].tile([P, KT, P], BF16, tag="qt", name="qt")
        if XSPLIT:
            h = KT // 2
            nc.sync.dma_start_transpose(qt[:, :h, :], qi[:, :h * P])
            nc.sync.dma_start_transpose(qt[:, h:, :], qi[:, h * P:])
        else:
            nc.sync.dma_start_transpose(qt[:], qi[:])
        qtss[(rep, u)] = qt
        if ti == grp - 1:  # last reader of lb issued
            lbs.pop((rep, g))

    def drain_one():
        ps, ob, ti, store = pending.pop(0)
        if DRAIN == "act" or (DRAIN == "alt" and ti % 2 == 0):
            nc.scalar.copy(ob[:, ti, :], ps[:])
        else:
            nc.vector.tensor_copy(ob[:, ti, :], ps[:])
        if store is not None:
            rep, g, ob_g = store
            _stq = {"gpsimd": nc.gpsimd, "scalar": nc.scalar,
                    "sync": nc.sync}[STQ]
            oview = out[g * grp * P:(g + 1) * grp * P, :].rearrange(
                "(t p) n -> p t n", p=P)
            if SSPLIT:
                h = grp // 2
                _stq.dma_start(oview[:, :h, :], ob_g[:, :h, :])
                _stq.dma_start(oview[:, h:, :], ob_g[:, h:, :])
            else:
                _stq.dma_start(oview, ob_g[:])

    def mm(rep, u):
        g, ti = divmod(u, grp)
        if ti == 0:
            obs[(rep, g)] = pl["lout"].tile([P, grp, N], FP16, tag="ob",
                                            name="ob")
        ob = obs[(rep, g)]
        qt = qtss[(rep, u)]
        # one 2-bank PSUM tile per m-tile; each matmul stays in one bank
        ps = pl["mpsum"].tile([P, NCH * NF], FP32, tag="ps", name="ps")
        for nj in range(NCH):
            for j in range(KT):
                nc.tensor.matmul(ps[:, nj * NF:(nj + 1) * NF],
                                 lhsT=qt[:, j, :],
                                 rhs=QRSs[rep][nj][:, j, :],
                                 start=(j == 0), stop=(j == KT - 1))
        last = ti == grp - 1
        pending.append((ps, ob, ti, (rep, g, ob) if last else None))
        while len(pending) > LAG:
            drain_one()
        if ti == grp - 1:
            obs.pop((rep, g))
        qtss.pop((rep, u))

    # ---------------- prologue (body 0 cold start) ----------------
    load_R(0)
    load_group(0, 0)
    load_group(0, 1)
    if RHS == "ar":
        for t in range(NPREP):
            rhs_ar(0, t)
    else:
        for t in range(NT):
            chain(0, t)
    load_group(0, 2)
    scales(0, 0)
    nq = 0
    for _ in range(QLOOK):
        quant(*divmod(nq, mt))
        nq += 1

    # ---------------- unified stream ----------------
    TT = repeats * mt
    for Ug in range(TT):
        rep, u = divmod(Ug, mt)
        g, ti = divmod(u, grp)
        def _scales_ahead():
            ns = Ug // grp + 1
            if ns < repeats * ng:
                nrep, ng_ = divmod(ns, ng)
                if (nrep, ng_) not in scs:
                    scales(nrep, ng_)

        def _housekeep():
            # loads GLOOK groups ahead
            nl = Ug // grp + GLOOK
            if nl < repeats * ng:
                lrep, lg = divmod(nl, ng)
                if (lrep, lg) not in lbs and all(
                        k != (lrep, lg) for k in scs):
                    load_group(lrep, lg)
        if ti == SCAT:
            _scales_ahead()
        if ti == 0 and not LATELOAD:
            _housekeep()
        if u == ((4 if ARSPREAD == 1 else 8) if RHS == "ar"
                 else mt // 2 - 4) and rep + 1 < repeats:
            load_R(rep + 1)
        # inject next body's rhs prep into this body's tail, one piece per
        # tile, finishing a few tiles before the boundary
        if RHS == "ar":
            if rep + 1 < repeats:
                st0 = mt - CLEAD - 6 if ARSPREAD == 2 else mt - CLEAD - 15
                k2 = u - st0
                if 0 <= k2:
                    for t in range(ARSPREAD * k2, ARSPREAD * (k2 + 1)):
                        if 0 <= t < NPREP:
                            rhs_ar(rep + 1, t)
        elif mt - CLEAD <= u < mt - CLEAD + NT and rep + 1 < repeats:
            chain(rep + 1, u - (mt - CLEAD))
        # quant runs QLOOK tiles ahead, +2 near the body boundary so the
        # next body's first x-bars clear the queue before they're needed
        boost = 2 if u >= mt - 4 else 0
        target = min(Ug + QLOOK + boost, TT - 1)
        while nq <= target:
            vrep, vu = divmod(nq, mt)
            if (vrep, vu // grp) not in scs:
                scales(vrep, vu // grp)
            quant(vrep, vu)
            nq += 1
        if ti == 0 and LATELOAD:
            _housekeep()
        mm(rep, u)
    while pending:
        drain_one()


_CACHE = {}


def _build(m_shard: int, repeats: int = 1, timing: bool = False) -> bacc.Bacc:
    key = (m_shard, repeats, timing)
    if key in _CACHE:
        return _CACHE[key]
    nc = bacc.Bacc("TRN2", target_bir_lowering=False, debug=False)
    lhs = nc.dram_tensor("lhs", [m_shard, K], FP32, kind="ExternalInput").ap()
    rhs = nc.dram_tensor("rhs", [K, N], FP32, kind="ExternalInput").ap()
    out = nc.dram_tensor("out", [m_shard, N], FP16, kind="ExternalOutput").ap()
    rhs_out = lhs_out = None
    if timing:
        # pass-through copies so timing loops can keep inputs device-resident
        rhs_out = nc.dram_tensor("rhs_out", [K, N], FP32,
                                 kind="ExternalOutput").ap()
        lhs_out = nc.dram_tensor("lhs_out", [m_shard, K], FP32,
                                 kind="ExternalOutput").ap()
    with tile.TileContext(nc) as tc:
        if rhs_out is not None:
            nc.scalar.dma_start(rhs_out[:], rhs[:])
            nc.scalar.dma_start(lhs_out[:], lhs[:])
        with ExitStack() as ctx:
            pools = [
                ("const", 1, "SBUF"),
                ("rhsq", 1, "SBUF"),
                ("qrs", 2, "SBUF"),
                ("scales", 8, "SBUF"),
                ("mpsum", 3, "PSUM"),
                ("rpsum", 2, "PSUM"),
                ("lload", 4, "SBUF"),
                ("lpass", 3, "SBUF"),
                ("lq", 4, "SBUF"),
                ("lqt", 8, "SBUF"),
                ("lout", 2, "SBUF"),
                ("rtrow", 3, "SBUF"),
                ("rquant", 2, "SBUF"),
            ] if RHS == "chain" else [
                ("const", 1, "SBUF"),
                ("rhsq", 1, "SBUF"),
                ("qrs", 2, "SBUF"),
                ("scales", 8, "SBUF"),
                ("mpsum", 4, "PSUM"),
                ("lload", 3, "SBUF"),
                ("lpass", 2, "SBUF"),
                ("lq", 3, "SBUF"),
                ("lqt", 6, "SBUF"),
                ("lout", 2, "SBUF"),
                ("arp", 2, "SBUF"),
                ("arm", 2, "SBUF"),
                ("arsc", 1, "SBUF"),
                ("arin", 1, "SBUF"),
                ("art1", 2, "SBUF"),
                ("art2", 2, "SBUF"),
                ("art3", 2, "SBUF"),
            ]
            pl = {
                name: ctx.enter_context(
                    tc.tile_pool(name=name, bufs=bufs, space=space))
                for name, bufs, space in pools
            }
            ident = pl["const"].tile([P, P], FP32)
            make_identity(nc, ident)
            _stream(tc, pl, ident, out, lhs, rhs, m_shard, repeats)
    nc.compile()
    _CACHE[key] = nc
    return nc


def kernel(lhs: np.ndarray, rhs: np.ndarray) -> np.ndarray:
    b, sq, k = lhs.shape
    lhs_flat = np.ascontiguousarray(lhs, dtype=np.float32).reshape(b * sq, k)
    rhs = np.ascontiguousarray(rhs, dtype=np.float32)
    m_shard = (b * sq) // N_CORES

    nc = _build(m_shard)
    in_maps = [
        {"lhs": lhs_flat[c * m_shard:(c + 1) * m_shard], "rhs": rhs}
        for c in range(N_CORES)
    ]
    res = run_bass_kernel_spmd(nc, in_maps, core_ids=list(range(N_CORES)))
    outs = [np.asarray(res.results[c]["out"], dtype=np.float32)
            for c in range(N_CORES)]
    return np.concatenate(outs, axis=0).reshape(b, sq, rhs.shape[1])

